# revision 25
# baseline (speedup 1.0000x reference)
"""CrossAttention (B=1, S=4096, H=8, DH=40) on 8 Trainium2 NeuronCores.

Sharding: tensor-parallel over the 8 heads — core h computes head h's full
attention plus its partial output projection; the host sums the 8 partials
and adds the bias.

Per-core dataflow (PE stays in 64-row tiling for the steady state; fp32
accumulation in PSUM; softmax renormalization cancels the systematic part
of the P quantization):
  qkA/qkB [104, 4096] = packed q/k projections   (PE, K=320 in 3 chunks)
  vT      [65, 4096]  = v rows 0..39, ones row 40, zeros 41..64
  vsb8    [128, jt, 65] fp8 = PE-transposed v' tiles
  ST      [128j, 512i] pairs: T0 row-tile j0, T8 row-tile j1   (PE, K=40)
  PT      = exp(ST/sqrt(40) - 4)    (ScalarE, PSUM->SBUF, fp8e4 out)
  O'.T|r  [65, 512i]: fp8 DoubleRow AV — T0 contracts the lower halves of
          both j-tiles of a group, T8 the upper halves  (PE, 2x fp8)
  o65     = avq0 + avq1  (DVE, bf16; row 40 = r, the softmax denominators)
  yp      [128, 321] = o65_s.T @ [WoT | e40]  (PE; col 320 = r)
  ysb     = yp[:, 0:320] * reciprocal(yp[:, 320])  (DVE per-partition)
"""

import os

import ml_dtypes
import numpy as np

import concourse.bass as bass
import concourse.mybir as mybir
from concourse import bass_utils, masks
from concourse.tile import TileContext

S = 4096
D = 320
H = 8
DH = 40
N_CORES = 8
CHUNK = 512               # i-chunk width (one fp32 PSUM bank)
VW = 65                   # v' width: 40 v cols, zeros, ones col 64 (64 is
                          # the only 32-aligned partition base the BIR
                          # verifier accepts for the ones memset; 65
                          # partitions also forces 128-col PE tiling)
GJ = 2                    # j-tiles per exp group (2 PSUM banks)
QKW = 104                 # packed q/k projection rows (q@0..39, k@64..103)
SCALE = float(DH) ** -0.5

# fp8 DoubleRow AV halves the AV streaming time but measured rel err is
# 2.3e-2 (over the 2e-2 gate); bf16 pairs measure 4.6e-3. Scalar exp is
# the bottleneck either way, so default to bf16.
AV_FP8 = bool(int(os.environ.get('AV_FP8', '0')))
EXPB = -4.0 if AV_FP8 else 0.0  # keeps max P ~ e^5.4 < fp8e4 max 448;
                                # cancels exactly in softmax normalization

F32 = mybir.dt.float32
BF16 = mybir.dt.bfloat16
FP8 = mybir.dt.float8e4
EXP = mybir.ActivationFunctionType.Exp
ADD = mybir.AluOpType.add
MULT = mybir.AluOpType.mult
DR = mybir.MatmulPerfMode.DoubleRow

_COMPILED = {}


def _split_sync_waits(nc, max_waits=1):
    """This walrus build rejects instructions with more than one sync wait.
    Spill the excess onto same-engine nops placed just before the
    instruction (engine streams execute in program order, so all waits are
    satisfied before the instruction issues)."""
    for f in nc.m.functions:
        for bb in f.blocks:
            out = []
            changed = False
            for inst in bb.instructions:
                si = inst.sync_info
                if si is not None and si.on_wait and len(si.on_wait) > max_waits:
                    waits = list(si.on_wait)
                    for i in range(max_waits, len(waits), max_waits):
                        nop = mybir.InstNoOp(
                            name=nc.get_next_instruction_name(),
                            engine=inst.engine,
                            bass_nofuse=True,
                            sync_info=mybir.SyncInfo(
                                on_wait=waits[i:i + max_waits], on_update=[]),
                        )
                        out.append(nop)
                    inst.sync_info = mybir.SyncInfo(
                        on_wait=waits[:max_waits],
                        on_update=list(si.on_update or []))
                    changed = True
                out.append(inst)
            if changed:
                bb.instructions = out


def _build(s=None, split=True):
    s = s or S
    n_chunks = s // CHUNK
    jt = s // 128
    ng = jt // GJ              # exp/AV groups per i-chunk
    nc = bass.Bass('TRN2', target_bir_lowering=False, debug=False)

    xT_d = nc.dram_tensor('xT', [D, s], BF16, kind='ExternalInput').ap()
    wq_d = nc.dram_tensor('wq', [D, DH], BF16, kind='ExternalInput').ap()
    wk_d = nc.dram_tensor('wk', [D, DH], BF16, kind='ExternalInput').ap()
    wv_d = nc.dram_tensor('wv', [D, DH], BF16, kind='ExternalInput').ap()
    woT_d = nc.dram_tensor('woT', [DH, D], BF16, kind='ExternalInput').ap()
    out_d = nc.dram_tensor('out', [s, D], F32, kind='ExternalOutput').ap()

    KCH = (128, 128, 64)  # K chunks of D=320

    with TileContext(nc) as tc:
        with tc.tile_pool(name='const', bufs=1) as cpool, \
             tc.tile_pool(name='big', bufs=1) as big, \
             tc.tile_pool(name='pt', bufs=8) as ptp, \
             tc.tile_pool(name='work', bufs=3) as wkp, \
             tc.tile_pool(name='ps_st', bufs=2, space='PSUM') as ps_st, \
             tc.tile_pool(name='ps_small', bufs=2, space='PSUM') as ps_small, \
             tc.tile_pool(name='ps_av', bufs=1, space='PSUM') as ps_av:

            # ---- constants & input DMA (sync queue: xT; scalar queue: W) --
            ident = cpool.tile([128, 128], BF16, tag='ident')
            masks.make_identity(nc, ident[:, :])

            # Combined projection stationaries for the 2x-packed QK^T:
            # wA = [wq | 0 | wk], wB = [wk | 0 | wq] (104 cols), giving
            # qkA = [q@0-39 | k@64-103] and qkB = [k@0-39 | q@64-103].
            wA = cpool.tile([128, 3 * QKW], BF16, tag='wA')
            wB = cpool.tile([128, 3 * QKW], BF16, tag='wB')
            nc.gpsimd.memset(wA[:, :], 0.0)
            nc.gpsimd.memset(wB[:, :], 0.0)
            wv_sb = cpool.tile([128, 3 * DH], BF16, tag='wv')
            # output projection stationary plus the r passthrough column:
            # rows 0..39 = Wo_h.T, rows 40..63 zero, col 320 = e_64 so
            # yp col 320 = r (row 64 of o65)
            woT_sb = cpool.tile([VW, D + 1], BF16, tag='woT')
            nc.gpsimd.memset(woT_sb[:, :], 0.0)
            nc.gpsimd.memset(woT_sb[64:VW, D:D + 1], 1.0)

            xt0 = big.tile([128, s], BF16, tag='xt0')
            xt1 = big.tile([128, s], BF16, tag='xt1')
            xt2 = big.tile([64, s], BF16, tag='xt2')
            xts = (xt0, xt1, xt2)
            # x chunks on the sync DMA queue, in consumption order
            for c in range(n_chunks):
                cs = slice(c * CHUNK, (c + 1) * CHUNK)
                nc.sync.dma_start(xt0[:, cs], xT_d[0:128, cs])
                nc.sync.dma_start(xt1[:, cs], xT_d[128:256, cs])
                nc.sync.dma_start(xt2[:, cs], xT_d[256:320, cs])
            # weights on the scalar DMA queue (idle until the first exp):
            # wB first (first qkB proj), then wA, wv, woT
            for c, kk in enumerate(KCH):
                o = sum(KCH[:c])
                nc.scalar.dma_start(wB[0:kk, c * QKW:c * QKW + DH],
                                    wk_d[o:o + kk, :])
                nc.scalar.dma_start(wB[0:kk, c * QKW + 64:c * QKW + QKW],
                                    wq_d[o:o + kk, :])
            for c, kk in enumerate(KCH):
                o = sum(KCH[:c])
                nc.scalar.dma_start(wA[0:kk, c * QKW:c * QKW + DH],
                                    wq_d[o:o + kk, :])
                nc.scalar.dma_start(wA[0:kk, c * QKW + 64:c * QKW + QKW],
                                    wk_d[o:o + kk, :])
            for c, kk in enumerate(KCH):
                o = sum(KCH[:c])
                nc.scalar.dma_start(wv_sb[0:kk, c * DH:(c + 1) * DH],
                                    wv_d[o:o + kk, :])
            nc.scalar.dma_start(woT_sb[0:DH, 0:D], woT_d)

            qkA = big.tile([QKW, s], BF16, tag='qkA')
            qkB = big.tile([QKW, s], BF16, tag='qkB')
            # v plus ones row 40 and zero rows 41..64: transposing yields
            # v' tiles with the row-sum column at 40 and zero padding that
            # widens the AV output to 65 partitions.
            vT = big.tile([VW, s], BF16, tag='vT')
            # zero rows 32..63 first (32-aligned base); phase 1 then
            # overwrites rows 0..39 with v, leaving 40..63 zero
            nc.gpsimd.memset(vT[32:VW - 1, :], 0.0)
            nc.gpsimd.memset(vT[VW - 1:VW, :], 1.0)
            vsb = big.tile([128, jt, VW], FP8 if AV_FP8 else BF16, tag='vsb')

            # ---- phase 1 helpers (all injected into chunk 0's slots) ----
            def proj(dst, w_sb, c, ww, copy):
                ps = ps_small.tile([QKW, CHUNK], F32, tag='small',
                                   name='psproj')
                for ci, kk in enumerate(KCH):
                    nc.tensor.matmul(
                        ps[0:ww, :],
                        w_sb[0:kk, ci * ww:(ci + 1) * ww],
                        xts[ci][0:kk, c * CHUNK:(c + 1) * CHUNK],
                        start=(ci == 0), stop=(ci == 2))
                copy(dst[:, c * CHUNK:(c + 1) * CHUNK], ps[0:ww, :])

            def transpose_v(j):
                tp = ps_small.tile([128, VW], BF16, tag='small', name='pstp')
                nc.tensor.transpose(tp[:, :], vT[:, j * 128:(j + 1) * 128],
                                    ident[0:VW, 0:VW])
                nc.vector.tensor_copy(vsb[:, j, :], tp[:, :])

            def chunk0_slot(g):
                # one qk projection per slot (even: qkB, odd: qkA, both a
                # chunk ahead of the STs that need them); odd slots also
                # run the v chunk + transposes just ahead of their AV
                if g % 2 == 0:
                    k = g // 2 + 2
                    if k < n_chunks:
                        proj(qkB, wB, k, QKW, nc.vector.tensor_copy)
                else:
                    k = g // 2 + 1
                    if k < n_chunks:
                        proj(qkA, wA, k, QKW, nc.vector.tensor_copy)
                    m = g // 2
                    proj(vT[0:DH, :], wv_sb, m, DH, nc.scalar.copy)
                    for j in range(4 * m, 4 * m + 4):
                        transpose_v(j)

            # chunk 0's first STs need the c0 projections (and qkB c1
            # before slot 2, since slot 0 only projects qkA c1)
            proj(qkB, wB, 0, QKW, nc.vector.tensor_copy)
            proj(qkA, wA, 0, QKW, nc.vector.tensor_copy)
            proj(qkB, wB, 1, QKW, nc.vector.tensor_copy)

            # ---- main loop over i-chunks ----
            tail_pe = []
            pend = []   # (emit_av, pt, g, merge_or_None), lag 2 slots
            for c in range(n_chunks):
                cs = slice(c * CHUNK, (c + 1) * CHUNK)
                avq_h = []
                def avq():
                    # lazy: first AV of this chunk pops two slots after the
                    # previous chunk's last AV + merge, and the PSUM banks
                    # must not be re-tagged before those retire
                    if not avq_h:
                        avq_h.extend(ps_av.tile([VW, CHUNK], F32,
                                                tag=f'av{q}', name=f'av{q}')
                                     for q in range(2))
                    return avq_h

                def emit_av(pt, g, avq=avq):
                    avq = avq()
                    if AV_FP8:
                        # fp8 DoubleRow: T0 contracts the lower 64 j-rows
                        # of both group j-tiles, T8 the upper 64; separate
                        # accumulators merged at chunk end.
                        for q in range(2):
                            nc.tensor.matmul(
                                avq[q][:, :],
                                vsb[64 * q:64 * (q + 1),
                                    GJ * g:GJ * (g + 1), :],
                                pt[64 * q:64 * (q + 1), :, :],
                                start=(g == 0), stop=(g == ng - 1),
                                perf_mode=DR, tile_position=(64 * q, 0))
                        return
                    # bf16: one K=64 pair per j-tile, row tiles T0/T8 run
                    # concurrently into their own accumulators
                    for t in range(GJ):
                        j = GJ * g + t
                        for q in range(2):
                            nc.tensor.matmul(
                                avq[q][:, :],
                                vsb[64 * q:64 * (q + 1), j, :],
                                pt[64 * q:64 * (q + 1), t, :],
                                start=(g == 0 and t == 0),
                                stop=(g == ng - 1 and t == GJ - 1),
                                tile_position=(64 * q, 0))

                for g in range(ng):
                    if c == 0:
                        chunk0_slot(g)
                    elif tail_pe and g >= 2:
                        # previous chunk's deferred output projection (its
                        # merge pops at slot 1, so the queue fills then)
                        tail_pe.pop(0)()
                    st = ps_st.tile([128, GJ, CHUNK], F32, tag='st')
                    j0, j1 = GJ * g, GJ * g + 1
                    nc.tensor.matmul(
                        st[:, 0, :],
                        qkB[0:DH, j0 * 128:(j0 + 1) * 128], qkA[0:DH, cs],
                        start=True, stop=True)
                    nc.tensor.matmul(
                        st[:, 1, :],
                        qkA[64:QKW, j1 * 128:(j1 + 1) * 128],
                        qkB[64:QKW, cs],
                        start=True, stop=True)
                    pt = ptp.tile([128, GJ, CHUNK],
                                  FP8 if AV_FP8 else BF16, tag='pt')
                    nc.scalar.activation(pt[:, :, :], st[:, :, :], EXP,
                                         scale=SCALE, bias=EXPB)
                    pend.append([emit_av, pt, g, None])
                    while len(pend) > 1:
                        fn, p_, g_, fin = pend.pop(0)
                        fn(p_, g_)
                        if fin is not None:
                            fin()
                def make_merge(c, avq):
                    def merge():
                        avq_t = avq()
                        # merge the 2 partial accumulators on DVE (bf16
                        # out; row 64 carries the softmax denominators r).
                        # Only one PSUM input per DVE op.
                        m1 = wkp.tile([VW, CHUNK], F32, tag='m1')
                        nc.vector.tensor_copy(m1[:, :], avq_t[0][:, :])
                        o65 = wkp.tile([VW, CHUNK], BF16, tag='o65')
                        nc.vector.tensor_tensor(
                            out=o65[:, :], in0=avq_t[1][:, :],
                            in1=m1[:, :], op=ADD)
                        tail_pe.clear()
                        tail_pe.extend(make_tail(c, o65))
                    return merge
                pend[-1][3] = make_merge(c, avq)  # holder fn

                # output projection; col 320 = r, divided out
                # per-partition. Deferred into the next chunk's slots so
                # it never sits between the last exp of this chunk and the
                # first ST of the next.
                def make_tail(c, o65):
                    def one(s2):
                        def emit():
                            st_i = c * (CHUNK // 128) + s2
                            yp = ps_small.tile([128, D + 1], F32,
                                               tag='small', name='psyp')
                            nc.tensor.matmul(yp[:, :],
                                             o65[:, s2 * 128:(s2 + 1) * 128],
                                             woT_sb[:, :],
                                             start=True, stop=True)
                            rec = wkp.tile([128, 1], F32, tag='rec')
                            nc.vector.reciprocal(rec[:, :], yp[:, D:D + 1])
                            ysb = wkp.tile([128, D], F32, tag='ysb')
                            nc.vector.tensor_scalar(
                                out=ysb[:, :], in0=yp[:, 0:D],
                                scalar1=rec[:, 0:1], scalar2=None, op0=MULT)
                            nc.sync.dma_start(
                                out_d[st_i * 128:(st_i + 1) * 128, :],
                                ysb[:, :])
                        return emit
                    return [one(s2) for s2 in range(CHUNK // 128)]
            while pend:
                fn, p_, g_, fin = pend.pop(0)
                fn(p_, g_)
                if fin is not None:
                    fin()
            for t in tail_pe:
                t()

    if split:
        _split_sync_waits(nc)
    return nc


def kernel(x, Wq, Wk, Wv, Wo, bo):
    x = np.asarray(x, dtype=np.float32)
    Wq = np.asarray(Wq, dtype=np.float32)
    Wk = np.asarray(Wk, dtype=np.float32)
    Wv = np.asarray(Wv, dtype=np.float32)
    Wo = np.asarray(Wo, dtype=np.float32)
    bo = np.asarray(bo, dtype=np.float32)

    if 'nc' not in _COMPILED:
        _COMPILED['nc'] = _build()
    nc = _COMPILED['nc']

    bf = ml_dtypes.bfloat16
    xT = np.ascontiguousarray(x.reshape(S, D).T).astype(bf)
    in_maps = []
    for h in range(N_CORES):
        sl = slice(h * DH, (h + 1) * DH)
        in_maps.append({
            'xT': xT,
            'wq': np.ascontiguousarray(Wq[sl, :].T).astype(bf),
            'wk': np.ascontiguousarray(Wk[sl, :].T).astype(bf),
            'wv': np.ascontiguousarray(Wv[sl, :].T).astype(bf),
            'woT': np.ascontiguousarray(Wo[:, sl].T).astype(bf),
        })

    trace = bool(os.environ.get('BASS_KERNEL_TRACE'))

    def _run():
        return bass_utils.run_bass_kernel_spmd(
            nc, in_maps, core_ids=list(range(N_CORES)), trace=trace,
            tmpdir=os.environ.get('BASS_KERNEL_TRACE_DIR') or None)

    try:
        res = _run()
    except Exception:
        # A previously crashed NEFF can leave the device unrecoverable; the
        # failed attempt clears it, so one retry is usually enough.
        res = _run()
    _COMPILED['last_res'] = res

    acc = res.results[0]['out'].astype(np.float32).copy()
    for h in range(1, N_CORES):
        acc += res.results[h]['out']
    acc += bo[None, :]
    return acc.reshape(1, S, D)
